# revision 7
# baseline (speedup 1.0000x reference)
"""Trainium2 Bass kernel for nn_Discriminator (attentional recent discriminator).

Math notes (derived from the module definition, hardcoded here):
  - The attention matmul result is deleted (torch sorts a size-1 dim, so the
    "top-5" indices are always 0); the output depends only on node_vec rows
    0 and N-1 of each batch element.
  - hidden_in rows 1..5 are all node_vec[:,0,:], so after the node MLP the
    five repeated u0 blocks contract against ta_w1 blocks 1..5; those blocks
    are PRE-SUMMED on the host, so stage 3 is 2 matmuls (last-block + summed).

Sharding: pure data parallel over batch, 32 batches/core on 8 cores.
Weights + tables replicated. Host only reshapes/casts inputs and packs
weights; all data-dependent compute runs on device.

Gather strategy (v3): instead of dma_indirect1d (1 offset/partition, ~1.4us
of serialized gpsimd time per 128 descriptors -> 5 instructions), use the
MoE gather primitive InstDMAGatherAnt (nc.gpsimd.dma_gather) which takes an
arbitrary index count in ONE instruction and, with transpose=True, writes
each gathered row as a COLUMN of the destination -- i.e. it emits K-major
layout directly, eliminating all PE transposes and PSUM round-trips.
  - slot table T16 [3200, 128] bf16: row 200*s + t = slot-s values of token
    t in cols 16*(s%8) .. +16, zeros elsewhere (slots 0,1 = positional
    encodings zero-padded 8->16, slots 2..15 = embedding rows). One
    1024-index gather (i = (s%8)*128 + (s//8)*64 + q) yields sT [128, 1024];
    three in-place binary folds on the DVE (cols 0:512 += 512:1024, ...)
    collapse the disjoint-support slot columns into the two 128-deep
    K-chunks -- every add has a zero operand elementwise, so the folds are
    EXACT in bf16.
  - lstm rows ride a second 128-index transpose-gather (64 real + 64 pad)
    producing K-chunks 2..5 directly.
Contraction dim = 768 = 16*16 + 512 lstm, node_w1 rows permuted/padded on
the host to match; node_b1 is folded into K-row 8 (a pe pad row forced to
1.0 in the slot table). Gather indices are int16, laid out on host as pure
reindex/cast of trees (index ARITHMETIC -- +200s, +b*200 -- runs on DVE).

Biases: b1 folded into stage-1 K; ta_b1/ff_b1/ff_b2/ts_b folded via an
extra all-ones partition row on the rhs tiles and a bias row in the packed
weights; b2/ta_b2 applied with the DVE tensor_scalar per-partition pointer.
All activations run on the DVE, so the scalar engine and its act-table
load are unused.

Precision: bf16 operands with fp32 PSUM accumulation.
"""

import ml_dtypes
import numpy as np

import concourse.bass as bass
import concourse.mybir as mybir
import concourse.tile as tile
from concourse import bacc
from concourse.bass_utils import run_bass_kernel_spmd

# problem constants (hardcoded per harness contract)
B, N, M = 256, 200, 200
EMB_DICT, EMB_DIM, POS_DIM, HID, LSTM_DIM, MAX_LEN, TOPK = 200, 16, 8, 32, 512, 200, 5
NODE_DIM = 2 * POS_DIM + 14 * EMB_DIM + LSTM_DIM  # 752

N_CORES = 8
NB = B // N_CORES  # 32 batches per core
NV = 2 * NB        # 64 node vectors per core (node 0 and node N-1)

NSLOT = 16                    # pe0, pe1, emb0..emb13
KDIM = 16 * NSLOT + LSTM_DIM  # 768
NCHUNK = KDIM // 128          # 6
T16_ROWS = NSLOT * EMB_DICT   # 3200
NSIDX = NSLOT * NV            # 1024 slot-gather indices
NLIDX = 128                   # lstm-gather indices (64 real + 64 pad)

F32 = mybir.dt.float32
BF16 = mybir.dt.bfloat16
I16 = mybir.dt.int16
NP_BF16 = ml_dtypes.bfloat16

# tin16 (i16) [128, 144] (16 rows replicated 8x) columns: 0:64 slot tokens t[q,s] (i = f*16+p),
# 64:128 slot bases 200*s, 128:136 lstm tokens t[q,16], 136:144 lstm row
# bases (q%32)*200
_TIN_COLS = 144

_WST1_COLS = NCHUNK * 128  # 768

# wsm (bf16) [128, 417] column layout
_C_W2 = 0        # rows 0:128, node_w2
_C_TA1L = 32     # rows 0:33, ta_w1 block 0 (multiplies u_last), row 32 zero
_C_TA1R = 160    # rows 0:33, sum of ta_w1 blocks 1..5, row 32 = ta_b1
_C_TAW2 = 288    # rows 0:128
_C_FFW1 = 320    # rows 0:33, row 32 = ff_b1
_C_FFW2 = 384    # rows 0:65, row 64 = ff_b2
_C_TSW = 416     # rows 0:33, row 32 = ts_b
_WSM_COLS = 417

ADD = mybir.AluOpType.add
MAX = mybir.AluOpType.max


def _pos_encoding():
    pos = np.arange(MAX_LEN, dtype=np.float32)[:, None]
    div = np.exp(
        np.arange(0, POS_DIM, 2, dtype=np.float32) * (-np.log(10000.0) / POS_DIM)
    )
    pe = np.zeros((MAX_LEN, POS_DIM), np.float32)
    pe[:, 0::2] = np.sin(pos * div)
    pe[:, 1::2] = np.cos(pos * div)
    return pe


def _ap3(tile_ap, dims):
    return bass.AP(tile_ap.tensor, tile_ap.offset, dims)


def build_nc():
    # Bacc (not plain Bass): its compile pass splits multi-wait sync into
    # InstEventSemaphore, which the walrus codegen requires (1 wait/inst).
    nc = bacc.Bacc(
        "TRN2",
        target_bir_lowering=False,
        debug=False,
        num_devices=N_CORES,
    )

    lstm = nc.dram_tensor("lstm", [NB * M, LSTM_DIM], BF16, kind="ExternalInput")
    tin_d = nc.dram_tensor("tin", [128, _TIN_COLS], I16, kind="ExternalInput")
    t16_d = nc.dram_tensor("t16", [T16_ROWS, 128], BF16, kind="ExternalInput")
    wst1_d = nc.dram_tensor("wst1", [128, _WST1_COLS], BF16, kind="ExternalInput")
    wsm_d = nc.dram_tensor("wsm", [128, _WSM_COLS], BF16, kind="ExternalInput")
    wfc_d = nc.dram_tensor("wfc", [HID, 2], F32, kind="ExternalInput")

    out_d = nc.dram_tensor("out", [1, NB], F32, kind="ExternalOutput")

    with tile.TileContext(nc) as tc:
        with (
            tc.tile_pool(name="sb", bufs=1) as sb,
            tc.tile_pool(name="ps", bufs=1, space="PSUM") as ps,
        ):
            # ---- input DMAs; tin16 heads the gather critical path ----
            tin = sb.tile([128, _TIN_COLS], I16, tag="tin")
            nc.scalar.dma_start(tin[:], tin_d[:])
            wst1 = sb.tile([128, _WST1_COLS], BF16, tag="wst1")
            nc.scalar.dma_start(wst1[:], wst1_d[:])
            wfc = sb.tile([HID, 2], F32, tag="wfc")
            nc.sync.dma_start(wfc[:], wfc_d[:])
            wsm = sb.tile([128, _WSM_COLS], BF16, tag="wsm")
            nc.sync.dma_start(wsm[:], wsm_d[:])

            # ---- index tiles; rows 16:128 only exist to satisfy the
            # [128, n/16] idx-AP shape the gather ucode expects ----
            sidx = sb.tile([128, NSIDX // 16], I16, tag="sidx")
            lidx = sb.tile([128, NLIDX // 16], I16, tag="lidx")

            # ---- ones rows for bias folding (off critical path) ----
            u = sb.tile([HID + 1, NV], BF16, tag="u")
            g2 = sb.tile([HID + 1, NB], BF16, tag="g2")
            g3 = sb.tile([2 * HID + 1, NB], BF16, tag="g3")
            g4 = sb.tile([HID + 1, NB], BF16, tag="g4")
            nc.gpsimd.memset(u[HID : HID + 1, :], 1.0)
            nc.gpsimd.memset(g2[HID : HID + 1, :], 1.0)
            nc.gpsimd.memset(g3[2 * HID : 2 * HID + 1, :], 1.0)
            nc.gpsimd.memset(g4[HID : HID + 1, :], 1.0)

            # ---- index math (DVE): token + base ----
            nc.vector.tensor_tensor(
                out=lidx[:], in0=tin[:, 128:136], in1=tin[:, 136:144], op=ADD,
            )
            nc.vector.tensor_tensor(
                out=sidx[:], in0=tin[:, 0:64], in1=tin[:, 64:128], op=ADD,
            )

            # ---- gathers: one InstDMAGatherAnt each, transpose=True writes
            # gathered rows as K-major columns ----
            lT = sb.tile([128, 4 * NLIDX], BF16, tag="lT")
            lt0 = lT[:]
            nc.gpsimd.dma_gather(
                _ap3(lt0, [lt0.ap[0], [NLIDX, 4], [1, NLIDX]]),
                lstm[:], lidx[:], NLIDX, NLIDX, LSTM_DIM, transpose=True,
            )
            sT = sb.tile([128, NSIDX], BF16, tag="sT")
            st0 = sT[:]
            nc.gpsimd.dma_gather(
                _ap3(st0, [st0.ap[0], [NSIDX, 1], [1, NSIDX]]),
                t16_d[:], sidx[:], NSIDX, NSIDX, 128, transpose=True, single_packet=False,
            )

            # ---- fold the 16 disjoint-support slot column-groups into the
            # two 128-deep K-chunks (exact in bf16: one operand is always 0)
            nc.vector.tensor_tensor(
                out=sT[:, 0:512], in0=sT[:, 0:512], in1=sT[:, 512:1024], op=ADD,
            )
            nc.vector.tensor_tensor(
                out=sT[:, 0:256], in0=sT[:, 0:256], in1=sT[:, 256:512], op=ADD,
            )
            nc.vector.tensor_tensor(
                out=sT[:, 0:128], in0=sT[:, 0:128], in1=sT[:, 128:256], op=ADD,
            )

            # ---- stage 1: h1 = relu(w1p.T @ v)  [128, NV]; b1 in K-row 8
            # lstm chunks 2..5 first (their gather lands first), slots last.
            h1p = ps.tile([128, NV], F32, tag="h1p")
            rhs_by_chunk = {
                0: sT[:, 0:NV], 1: sT[:, NV : 2 * NV],
                2: lT[:, 0:NV], 3: lT[:, 128 : 128 + NV],
                4: lT[:, 256 : 256 + NV], 5: lT[:, 384 : 384 + NV],
            }
            chunk_order = [2, 3, 4, 5, 0, 1]
            for j, c in enumerate(chunk_order):
                nc.tensor.matmul(
                    h1p[:],
                    lhsT=wst1[:, bass.ts(c, 128)],
                    rhs=rhs_by_chunk[c],
                    start=(j == 0), stop=(j == NCHUNK - 1),
                )
            h1 = sb.tile([128, NV], BF16, tag="h1")
            nc.vector.tensor_scalar_max(h1[:], h1p[:], 0.0)

            # ---- stage 2: u = relu(w2.T @ h1 + b2)  [32, NV] ----
            up = ps.tile([HID, NV], F32, tag="small_p")
            nc.tensor.matmul(
                up[:], lhsT=wsm[:, _C_W2 : _C_W2 + HID], rhs=h1[:],
                start=True, stop=True,
            )
            nc.vector.tensor_scalar(
                out=u[0:HID, :], in0=up[:], scalar1=wfc[:, 0:1], scalar2=0.0,
                op0=ADD, op1=MAX,
            )

            # ---- stage 3: g1 = relu(ta1l.T @ u_last + ta1r.T @ u_0 + tab1)
            # cols 0:NB of u are node0, NB:NV are nodeL; tab1 rides row 32 of
            # the presummed block against u's ones row.
            g1p = ps.tile([128, NB], F32, tag="mid_p")
            nc.tensor.matmul(
                g1p[:], lhsT=wsm[0 : HID + 1, _C_TA1L : _C_TA1L + 128],
                rhs=u[:, NB:NV], start=True, stop=False,
            )
            nc.tensor.matmul(
                g1p[:], lhsT=wsm[0 : HID + 1, _C_TA1R : _C_TA1R + 128],
                rhs=u[:, 0:NB], start=False, stop=True,
            )
            g1 = sb.tile([128, NB], BF16, tag="g1")
            nc.vector.tensor_scalar_max(g1[:], g1p[:], 0.0)

            # ---- stage 4: g2 = relu(taw2.T @ g1 + tab2)  [32, NB] ----
            g2p = ps.tile([HID, NB], F32, tag="small_p")
            nc.tensor.matmul(
                g2p[:], lhsT=wsm[:, _C_TAW2 : _C_TAW2 + HID], rhs=g1[:],
                start=True, stop=True,
            )
            nc.vector.tensor_scalar(
                out=g2[0:HID, :], in0=g2p[:], scalar1=wfc[:, 1:2], scalar2=0.0,
                op0=ADD, op1=MAX,
            )

            # ---- stage 5: g3 = relu(ffw1.T @ g2)  [64, NB]; ffb1 in row 32
            g3p = ps.tile([2 * HID, NB], F32, tag="mid_p")
            nc.tensor.matmul(
                g3p[:], lhsT=wsm[0 : HID + 1, _C_FFW1 : _C_FFW1 + 2 * HID],
                rhs=g2[:], start=True, stop=True,
            )
            nc.vector.tensor_scalar_max(g3[0 : 2 * HID, :], g3p[:], 0.0)

            # ---- stage 6: g4 = relu(ffw2.T @ g3)  [32, NB]; ffb2 in row 64
            g4p = ps.tile([HID, NB], F32, tag="small_p")
            nc.tensor.matmul(
                g4p[:], lhsT=wsm[0 : 2 * HID + 1, _C_FFW2 : _C_FFW2 + HID],
                rhs=g3[:], start=True, stop=True,
            )
            nc.vector.tensor_scalar_max(g4[0:HID, :], g4p[:], 0.0)

            # ---- stage 7: out = tsw.T @ g4  [1, NB]; ts_b in row 32 ----
            op_ = ps.tile([1, NB], F32, tag="small_p")
            nc.tensor.matmul(
                op_[:], lhsT=wsm[0 : HID + 1, _C_TSW : _C_TSW + 1], rhs=g4[:],
                start=True, stop=True,
            )
            o = sb.tile([1, NB], F32, tag="o")
            nc.vector.tensor_copy(o[:], op_[:])
            nc.sync.dma_start(out_d[:], o[:])

    nc.finalize()
    return nc


def _slot_rows(inputs):
    """16 lookup tables, each [200, 16] f32 (pe slots zero-padded)."""
    emb = np.asarray(inputs["emb"], np.float32).reshape(EMB_DICT, EMB_DIM)
    pe = _pos_encoding()
    rows = []
    for k in range(2):
        r = np.zeros((EMB_DICT, 16), np.float32)
        r[:, 0:POS_DIM] = pe
        rows.append(r)
    for _ in range(14):
        rows.append(emb)
    return rows


def _pack_t16(inputs):
    rows = _slot_rows(inputs)
    t16 = np.zeros((T16_ROWS, 128), np.float32)
    for s in range(NSLOT):
        band = 16 * (s % 8)
        t16[EMB_DICT * s : EMB_DICT * (s + 1), band : band + 16] = rows[s]
    # slot-0 pad col 8 carries the constant 1 that multiplies node_b1
    t16[0:EMB_DICT, 8] = 1.0
    return t16.astype(NP_BF16)


def _pack_weights(inputs):
    def w(name, shape):
        return np.asarray(inputs[name], np.float32).reshape(shape)

    # permute/zero-pad node_w1 rows to the padded 768 contraction order
    w1 = w("node_w1", (NODE_DIM, 4 * HID))
    w1p = np.zeros((KDIM, 4 * HID), np.float32)
    w1p[0:POS_DIM] = w1[0:POS_DIM]                      # slot 0: pe(t0)
    w1p[16 : 16 + POS_DIM] = w1[POS_DIM : 2 * POS_DIM]  # slot 1: pe(t1)
    for j in range(14):                                 # slots 2..15: emb
        w1p[16 * (2 + j) : 16 * (2 + j) + EMB_DIM] = (
            w1[2 * POS_DIM + EMB_DIM * j : 2 * POS_DIM + EMB_DIM * (j + 1)]
        )
    w1p[16 * NSLOT :] = w1[2 * POS_DIM + 14 * EMB_DIM :]  # lstm block
    w1p[8] = w("node_b1", (4 * HID,))                     # b1 on the ones row

    wst1 = np.zeros((128, _WST1_COLS), np.float32)
    for c in range(NCHUNK):
        wst1[:, 128 * c : 128 * (c + 1)] = w1p[128 * c : 128 * (c + 1), :]

    wsm = np.zeros((128, _WSM_COLS), np.float32)
    wsm[:, _C_W2 : _C_W2 + HID] = w("node_w2", (4 * HID, HID))
    taw1 = w("ta_w1", (6 * HID, 4 * HID))
    wsm[0:HID, _C_TA1L : _C_TA1L + 128] = taw1[0:HID]
    wsm[0:HID, _C_TA1R : _C_TA1R + 128] = taw1[HID:].reshape(5, HID, 128).sum(0)
    wsm[HID, _C_TA1R : _C_TA1R + 128] = w("ta_b1", (4 * HID,))
    wsm[:, _C_TAW2 : _C_TAW2 + HID] = w("ta_w2", (4 * HID, HID))
    wsm[0:HID, _C_FFW1 : _C_FFW1 + 2 * HID] = w("ff_w1", (HID, 2 * HID))
    wsm[HID, _C_FFW1 : _C_FFW1 + 2 * HID] = w("ff_b1", (2 * HID,))
    wsm[0 : 2 * HID, _C_FFW2 : _C_FFW2 + HID] = w("ff_w2", (2 * HID, HID))
    wsm[2 * HID, _C_FFW2 : _C_FFW2 + HID] = w("ff_b2", (HID,))
    wsm[0:HID, _C_TSW] = w("ts_w", (HID,))
    wsm[HID, _C_TSW] = w("ts_b", (1,))[0]

    wfc = np.zeros((HID, 2), np.float32)
    wfc[:, 0] = w("node_b2", (HID,))
    wfc[:, 1] = w("ta_b2", (HID,))
    return wst1.astype(NP_BF16), wsm.astype(NP_BF16), wfc


def _make_tin16(t2):
    """t2: [NV, 17] tokens (node0 b0..31 | nodeL b0..31).  Pure reindex of
    trees plus input-independent constant columns; the adds run on-device."""
    tin = np.zeros((16, _TIN_COLS), np.int16)
    f = np.arange(NV)
    s_of_f = 8 * ((f // 4) % 2) + f // 8          # slot for index block f
    p = np.arange(16)
    q_of_pf = 16 * (f[None, :] % 4) + p[:, None]  # node-vector for (p, f)
    tin[:, 0:NV] = t2[q_of_pf, s_of_f[None, :]]
    tin[:, NV : 2 * NV] = (EMB_DICT * s_of_f)[None, :]
    f2 = np.arange(8)
    i2 = f2[None, :] * 16 + p[:, None]            # lstm gather index
    real = i2 < NV
    tin[:, 128:136] = np.where(real, t2[np.minimum(i2, NV - 1), 16], 0)
    tin[:, 136:144] = np.where(real, (i2 % NB) * M, 0)
    # the gather ucode's Q7 cores each read their own 16-partition stripe of
    # the [128, n/16] index tile; replicate the 16 rows across all 8 stripes
    return np.tile(tin, (8, 1))


def make_in_maps(inputs):
    lstm = np.asarray(inputs["lstm_out_list"], np.float32).astype(NP_BF16)
    trees = np.asarray(inputs["trees"]).astype(np.int32)

    wst1, wsm, wfc = _pack_weights(inputs)
    shared = {
        "t16": _pack_t16(inputs),
        "wst1": wst1,
        "wsm": wsm,
        "wfc": wfc,
    }
    in_maps = []
    for c in range(N_CORES):
        sl = slice(c * NB, (c + 1) * NB)
        # [64, 17]: rows 0:32 = node 0 of each batch, rows 32:64 = node N-1
        t2 = trees[sl][:, [0, N - 1], :].transpose(1, 0, 2).reshape(NV, 17)
        in_maps.append(
            {
                "lstm": np.ascontiguousarray(lstm[sl].reshape(NB * M, LSTM_DIM)),
                "tin": _make_tin16(t2),
                **shared,
            }
        )
    return in_maps


_NC_CACHE = None


def run_on_hw(inputs, **kwargs):
    global _NC_CACHE
    if _NC_CACHE is None:
        _NC_CACHE = build_nc()
    in_maps = make_in_maps(inputs)
    return run_bass_kernel_spmd(
        _NC_CACHE, in_maps, core_ids=list(range(N_CORES)), **kwargs
    )


def kernel(**inputs) -> np.ndarray:
    res = run_on_hw(inputs)
    out = np.empty((B, 1), np.float32)
    for c in range(N_CORES):
        out[c * NB : (c + 1) * NB, 0] = res.results[c]["out"][0]
    return out


# revision 8
# speedup vs baseline: 1.6255x; 1.6255x over previous
"""Trainium2 Bass kernel for nn_Discriminator (attentional recent discriminator).

Math notes (derived from the module definition, hardcoded here):
  - The attention matmul result is deleted (torch sorts a size-1 dim, so the
    "top-5" indices are always 0); the output depends only on node_vec rows
    0 and N-1 of each batch element.
  - hidden_in rows 1..5 are all node_vec[:,0,:], so after the node MLP the
    five repeated u0 blocks contract against ta_w1 blocks 1..5; those blocks
    are PRE-SUMMED on the host, so stage 3 is 2 matmuls (last-block + summed).

Sharding: pure data parallel over batch, 32 batches/core on 8 cores.
Weights + tables replicated. Host only reshapes/casts inputs and packs
weights; all data-dependent arithmetic runs on device.

Gather strategy (v4): SWDGE descriptor generation AND the SDMA drain both
cost ~9-10ns per descriptor, so the 1024 small per-(node,slot) lookups must
not ride DMA at all.  Instead the 16 tiny lookup tables live in SBUF as an
f32 table tbl[128, 400] (partition k holds the 200-entry table for slot
k//16 (+8 for cols 200:400), element k%16) and ONE gpsimd ap_gather
instruction (~0.4us on the 8 Q7 cores, zero DMA descriptors; its
per-16-partition-group index semantics match slots exactly, and the indices
are plain token values laid out by the host) produces both 128-deep
K-chunks [128, 128] in one shot.  A single DVE copy converts f32->bf16 for
the PE.  Only the 64 lstm rows (512 wide) still use an indirect DMA
(64 descriptors) + 4 PE transposes.  Contraction dim = 768 = 16*16 + 512
lstm, node_w1 rows permuted/padded on the host to match; node_b1 is folded
into K-row 8 (a pe pad row forced to 1.0 in the slot table).

Biases: b1 folded into stage-1 K; ta_b1/ff_b1/ff_b2/ts_b folded via an
extra all-ones partition row on the rhs tiles and a bias row in the packed
weights; b2/ta_b2 applied with the DVE tensor_scalar per-partition pointer.
All activations run on the DVE, so the scalar engine and its act-table
load are unused.

Precision: bf16 operands with fp32 PSUM accumulation.
"""

import ml_dtypes
import numpy as np

import concourse.bass as bass
import concourse.mybir as mybir
import concourse.tile as tile
from concourse import bacc
from concourse.bass import IndirectOffsetOnAxis
from concourse.bass_utils import run_bass_kernel_spmd
from concourse.library_config import ap_gather as ap_gather_lib

# problem constants (hardcoded per harness contract)
B, N, M = 256, 200, 200
EMB_DICT, EMB_DIM, POS_DIM, HID, LSTM_DIM, MAX_LEN, TOPK = 200, 16, 8, 32, 512, 200, 5
NODE_DIM = 2 * POS_DIM + 14 * EMB_DIM + LSTM_DIM  # 752

N_CORES = 8
NB = B // N_CORES  # 32 batches per core
NV = 2 * NB        # 64 node vectors per core (node 0 and node N-1)

NSLOT = 16                    # pe0, pe1, emb0..emb13
KDIM = 16 * NSLOT + LSTM_DIM  # 768
NCHUNK = KDIM // 128          # 6

F32 = mybir.dt.float32
BF16 = mybir.dt.bfloat16
I16 = mybir.dt.int16
I32 = mybir.dt.int32
NP_BF16 = ml_dtypes.bfloat16

# wst1 (bf16) [128, 832]: stage-1 weight chunks at 128c, identity at 768
_C_IDENT = NCHUNK * 128  # 768
_WST1_COLS = 832

# wsm (bf16) [128, 417] column layout
_C_W2 = 0        # rows 0:128, node_w2
_C_TA1L = 32     # rows 0:33, ta_w1 block 0 (multiplies u_last), row 32 zero
_C_TA1R = 160    # rows 0:33, sum of ta_w1 blocks 1..5, row 32 = ta_b1
_C_TAW2 = 288    # rows 0:128
_C_FFW1 = 320    # rows 0:33, row 32 = ff_b1
_C_FFW2 = 384    # rows 0:65, row 64 = ff_b2
_C_TSW = 416     # rows 0:33, row 32 = ts_b
_WSM_COLS = 417

ADD = mybir.AluOpType.add
MAX = mybir.AluOpType.max


def _pos_encoding():
    pos = np.arange(MAX_LEN, dtype=np.float32)[:, None]
    div = np.exp(
        np.arange(0, POS_DIM, 2, dtype=np.float32) * (-np.log(10000.0) / POS_DIM)
    )
    pe = np.zeros((MAX_LEN, POS_DIM), np.float32)
    pe[:, 0::2] = np.sin(pos * div)
    pe[:, 1::2] = np.cos(pos * div)
    return pe


def build_nc():
    # Bacc (not plain Bass): its compile pass splits multi-wait sync into
    # InstEventSemaphore, which the walrus codegen requires (1 wait/inst).
    nc = bacc.Bacc(
        "TRN2",
        target_bir_lowering=False,
        debug=False,
        num_devices=N_CORES,
    )

    lstm = nc.dram_tensor("lstm", [NB * M, LSTM_DIM], BF16, kind="ExternalInput")
    tok_d = nc.dram_tensor("tok", [128, 8], I16, kind="ExternalInput")
    li_d = nc.dram_tensor("li", [NV, 2], I32, kind="ExternalInput")
    tbl_d = nc.dram_tensor("tbl", [128, 2 * EMB_DICT], F32, kind="ExternalInput")
    wst1_d = nc.dram_tensor("wst1", [128, _WST1_COLS], BF16, kind="ExternalInput")
    wsm_d = nc.dram_tensor("wsm", [128, _WSM_COLS], BF16, kind="ExternalInput")
    wfc_d = nc.dram_tensor("wfc", [HID, 2], F32, kind="ExternalInput")

    out_d = nc.dram_tensor("out", [1, NB], F32, kind="ExternalOutput")

    with tile.TileContext(nc) as tc:
        with (
            tc.tile_pool(name="sb", bufs=1) as sb,
            tc.tile_pool(name="pst", bufs=3, space="PSUM") as pst,
            tc.tile_pool(name="ps", bufs=1, space="PSUM") as ps,
        ):
            # the ap_gather Q7 library load (~3.5us) overlaps the input DMAs
            nc.gpsimd.load_library(ap_gather_lib)

            # ---- input DMAs; tok/li head the gather critical path ----
            tok = sb.tile([128, 8], I16, tag="tok")
            nc.scalar.dma_start(tok[:], tok_d[:])
            li = sb.tile([NV, 2], I32, tag="li")
            nc.sync.dma_start(li[:], li_d[:])
            tbl = sb.tile([128, 2 * EMB_DICT], F32, tag="tbl")
            nc.scalar.dma_start(tbl[:], tbl_d[:])
            wsm = sb.tile([128, _WSM_COLS], BF16, tag="wsm")
            nc.sync.dma_start(wsm[:], wsm_d[:])
            wst1 = sb.tile([128, _WST1_COLS], BF16, tag="wst1")
            nc.scalar.dma_start(wst1[:], wst1_d[:])
            wfc = sb.tile([HID, 2], F32, tag="wfc")
            nc.sync.dma_start(wfc[:], wfc_d[:])

            # ---- ones rows for bias folding (off critical path) ----
            u = sb.tile([HID + 1, NV], BF16, tag="u")
            g2 = sb.tile([HID + 1, NB], BF16, tag="g2")
            g3 = sb.tile([2 * HID + 1, NB], BF16, tag="g3")
            g4 = sb.tile([HID + 1, NB], BF16, tag="g4")
            nc.gpsimd.memset(u[HID : HID + 1, :], 1.0)
            nc.gpsimd.memset(g2[HID : HID + 1, :], 1.0)
            nc.gpsimd.memset(g3[2 * HID : 2 * HID + 1, :], 1.0)
            nc.gpsimd.memset(g4[HID : HID + 1, :], 1.0)

            # ---- index math (DVE) ----
            lidx = sb.tile([NV, 1], I32, tag="lidx")
            nc.vector.tensor_tensor(
                out=lidx[:], in0=li[:, 0:1], in1=li[:, 1:2], op=ADD,
            )
            # chunk-1 token columns address table cols 200:400
            nc.vector.tensor_scalar_add(tok[:, 4:8], tok[:, 4:8], EMB_DICT)

            # ---- lstm gather: 64 descriptors on the gpsimd dynamic queue
            nvL = sb.tile([NV, LSTM_DIM], BF16, tag="nvL")
            nc.gpsimd.indirect_dma_start(
                out=nvL[:], out_offset=None, in_=lstm[:],
                in_offset=IndirectOffsetOnAxis(ap=lidx[:, 0:1], axis=0),
            )

            # ---- slot gather: ONE ap_gather emits both K-chunks ----
            sg = sb.tile([128, 2 * NV], F32, tag="sg")
            nc.gpsimd.ap_gather(
                sg[:], tbl[:], tok[:],
                channels=128, num_elems=2 * EMB_DICT, d=1, num_idxs=2 * NV,
            )
            vTs = sb.tile([128, 2 * NV], BF16, tag="vTs")
            nc.vector.tensor_copy(vTs[:], sg[:])

            # ---- lstm transposes into K-major chunks 2..5 ----
            ident = wst1[0:NV, _C_IDENT : _C_IDENT + NV]
            vTl = sb.tile([128, 4 * NV], BF16, tag="vTl")
            for c in range(4):
                ptt = pst.tile([128, NV], BF16, tag="ptt")
                nc.tensor.transpose(ptt[:], nvL[:, 128 * c : 128 * (c + 1)], ident)
                nc.vector.tensor_copy(vTl[:, bass.ts(c, NV)], ptt[:])

            # ---- stage 1: h1 = relu(w1p.T @ v)  [128, NV]; b1 in K-row 8
            # slot chunks 0,1 first (their data lands first), lstm last.
            h1p = ps.tile([128, NV], F32, tag="h1p")
            rhs_by_chunk = {
                0: vTs[:, 0:NV], 1: vTs[:, NV : 2 * NV],
                2: vTl[:, 0:NV], 3: vTl[:, NV : 2 * NV],
                4: vTl[:, 2 * NV : 3 * NV], 5: vTl[:, 3 * NV : 4 * NV],
            }
            chunk_order = [0, 1, 2, 3, 4, 5]
            for j, c in enumerate(chunk_order):
                nc.tensor.matmul(
                    h1p[:],
                    lhsT=wst1[:, bass.ts(c, 128)],
                    rhs=rhs_by_chunk[c],
                    start=(j == 0), stop=(j == NCHUNK - 1),
                )
            h1 = sb.tile([128, NV], BF16, tag="h1")
            nc.vector.tensor_scalar_max(h1[:], h1p[:], 0.0)

            # ---- stage 2: u = relu(w2.T @ h1 + b2)  [32, NV] ----
            up = ps.tile([HID, NV], F32, tag="small_p")
            nc.tensor.matmul(
                up[:], lhsT=wsm[:, _C_W2 : _C_W2 + HID], rhs=h1[:],
                start=True, stop=True,
            )
            nc.vector.tensor_scalar(
                out=u[0:HID, :], in0=up[:], scalar1=wfc[:, 0:1], scalar2=0.0,
                op0=ADD, op1=MAX,
            )

            # ---- stage 3: g1 = relu(ta1l.T @ u_last + ta1r.T @ u_0 + tab1)
            # cols 0:NB of u are node0, NB:NV are nodeL; tab1 rides row 32 of
            # the presummed block against u's ones row.
            g1p = ps.tile([128, NB], F32, tag="mid_p")
            nc.tensor.matmul(
                g1p[:], lhsT=wsm[0 : HID + 1, _C_TA1L : _C_TA1L + 128],
                rhs=u[:, NB:NV], start=True, stop=False,
            )
            nc.tensor.matmul(
                g1p[:], lhsT=wsm[0 : HID + 1, _C_TA1R : _C_TA1R + 128],
                rhs=u[:, 0:NB], start=False, stop=True,
            )
            g1 = sb.tile([128, NB], BF16, tag="g1")
            nc.vector.tensor_scalar_max(g1[:], g1p[:], 0.0)

            # ---- stage 4: g2 = relu(taw2.T @ g1 + tab2)  [32, NB] ----
            g2p = ps.tile([HID, NB], F32, tag="small_p")
            nc.tensor.matmul(
                g2p[:], lhsT=wsm[:, _C_TAW2 : _C_TAW2 + HID], rhs=g1[:],
                start=True, stop=True,
            )
            nc.vector.tensor_scalar(
                out=g2[0:HID, :], in0=g2p[:], scalar1=wfc[:, 1:2], scalar2=0.0,
                op0=ADD, op1=MAX,
            )

            # ---- stage 5: g3 = relu(ffw1.T @ g2)  [64, NB]; ffb1 in row 32
            g3p = ps.tile([2 * HID, NB], F32, tag="mid_p")
            nc.tensor.matmul(
                g3p[:], lhsT=wsm[0 : HID + 1, _C_FFW1 : _C_FFW1 + 2 * HID],
                rhs=g2[:], start=True, stop=True,
            )
            nc.vector.tensor_scalar_max(g3[0 : 2 * HID, :], g3p[:], 0.0)

            # ---- stage 6: g4 = relu(ffw2.T @ g3)  [32, NB]; ffb2 in row 64
            g4p = ps.tile([HID, NB], F32, tag="small_p")
            nc.tensor.matmul(
                g4p[:], lhsT=wsm[0 : 2 * HID + 1, _C_FFW2 : _C_FFW2 + HID],
                rhs=g3[:], start=True, stop=True,
            )
            nc.vector.tensor_scalar_max(g4[0:HID, :], g4p[:], 0.0)

            # ---- stage 7: out = tsw.T @ g4  [1, NB]; ts_b in row 32 ----
            op_ = ps.tile([1, NB], F32, tag="small_p")
            nc.tensor.matmul(
                op_[:], lhsT=wsm[0 : HID + 1, _C_TSW : _C_TSW + 1], rhs=g4[:],
                start=True, stop=True,
            )
            o = sb.tile([1, NB], F32, tag="o")
            nc.vector.tensor_copy(o[:], op_[:])
            nc.sync.dma_start(out_d[:], o[:])

    nc.finalize()
    return nc


def _slot_rows(inputs):
    """16 lookup tables, each [200, 16] f32 (pe slots zero-padded)."""
    emb = np.asarray(inputs["emb"], np.float32).reshape(EMB_DICT, EMB_DIM)
    pe = _pos_encoding()
    rows = []
    for k in range(2):
        r = np.zeros((EMB_DICT, 16), np.float32)
        r[:, 0:POS_DIM] = pe
        rows.append(r)
    for _ in range(14):
        rows.append(emb)
    return rows


def _pack_tbl(inputs):
    """[128, 400] f32: partition k = (slot k//16 [+8 for cols 200:400],
    elem k%16)."""
    rows = _slot_rows(inputs)
    # bf16-round the entries so downstream bf16 matmul operands are exact
    rows = [r.astype(NP_BF16).astype(np.float32) for r in rows]
    tbl = np.zeros((128, 2 * EMB_DICT), np.float32)
    for half in range(2):
        for s8 in range(8):
            for e in range(16):
                k = 16 * s8 + e
                tbl[k, EMB_DICT * half : EMB_DICT * (half + 1)] = rows[
                    8 * half + s8
                ][:, e]
    # slot-0 pad elem 8 carries the constant 1 that multiplies node_b1
    tbl[8, 0:EMB_DICT] = 1.0
    return tbl


def _pack_weights(inputs):
    def w(name, shape):
        return np.asarray(inputs[name], np.float32).reshape(shape)

    # permute/zero-pad node_w1 rows to the padded 768 contraction order
    w1 = w("node_w1", (NODE_DIM, 4 * HID))
    w1p = np.zeros((KDIM, 4 * HID), np.float32)
    w1p[0:POS_DIM] = w1[0:POS_DIM]                      # slot 0: pe(t0)
    w1p[16 : 16 + POS_DIM] = w1[POS_DIM : 2 * POS_DIM]  # slot 1: pe(t1)
    for j in range(14):                                 # slots 2..15: emb
        w1p[16 * (2 + j) : 16 * (2 + j) + EMB_DIM] = (
            w1[2 * POS_DIM + EMB_DIM * j : 2 * POS_DIM + EMB_DIM * (j + 1)]
        )
    w1p[16 * NSLOT :] = w1[2 * POS_DIM + 14 * EMB_DIM :]  # lstm block
    w1p[8] = w("node_b1", (4 * HID,))                     # b1 on the ones row

    wst1 = np.zeros((128, _WST1_COLS), np.float32)
    for c in range(NCHUNK):
        wst1[:, 128 * c : 128 * (c + 1)] = w1p[128 * c : 128 * (c + 1), :]
    wst1[0:NV, _C_IDENT : _C_IDENT + NV] = np.eye(NV, dtype=np.float32)

    wsm = np.zeros((128, _WSM_COLS), np.float32)
    wsm[:, _C_W2 : _C_W2 + HID] = w("node_w2", (4 * HID, HID))
    taw1 = w("ta_w1", (6 * HID, 4 * HID))
    wsm[0:HID, _C_TA1L : _C_TA1L + 128] = taw1[0:HID]
    wsm[0:HID, _C_TA1R : _C_TA1R + 128] = taw1[HID:].reshape(5, HID, 128).sum(0)
    wsm[HID, _C_TA1R : _C_TA1R + 128] = w("ta_b1", (4 * HID,))
    wsm[:, _C_TAW2 : _C_TAW2 + HID] = w("ta_w2", (4 * HID, HID))
    wsm[0:HID, _C_FFW1 : _C_FFW1 + 2 * HID] = w("ff_w1", (HID, 2 * HID))
    wsm[HID, _C_FFW1 : _C_FFW1 + 2 * HID] = w("ff_b1", (2 * HID,))
    wsm[0 : 2 * HID, _C_FFW2 : _C_FFW2 + HID] = w("ff_w2", (2 * HID, HID))
    wsm[2 * HID, _C_FFW2 : _C_FFW2 + HID] = w("ff_b2", (HID,))
    wsm[0:HID, _C_TSW] = w("ts_w", (HID,))
    wsm[HID, _C_TSW] = w("ts_b", (1,))[0]

    wfc = np.zeros((HID, 2), np.float32)
    wfc[:, 0] = w("node_b2", (HID,))
    wfc[:, 1] = w("ta_b2", (HID,))
    return wst1.astype(NP_BF16), wsm.astype(NP_BF16), wfc


def _make_tok(t2):
    """t2: [NV, 17] tokens (node0 b0..31 | nodeL b0..31).  ap_gather index
    tile [128, 8] i16: index j (j<64: chunk-0 col j; j>=64: chunk-1 col
    j-64) is read from (partition 16*g + j%16, col j//16) of partition
    group g; group g serves slot g (chunk 0) or 8+g (chunk 1).  Values are
    raw tokens -- pure reindexing of trees; the chunk-1 +200 column offset
    is added on the DVE."""
    tok = np.zeros((128, 8), np.int16)
    for g in range(8):
        for c in range(8):
            slot = g + (8 if c >= 4 else 0)
            for p0 in range(16):
                q = 16 * (c % 4) + p0
                tok[16 * g + p0, c] = t2[q, slot]
    return tok


def make_in_maps(inputs):
    lstm = np.asarray(inputs["lstm_out_list"], np.float32).astype(NP_BF16)
    trees = np.asarray(inputs["trees"]).astype(np.int32)

    wst1, wsm, wfc = _pack_weights(inputs)
    shared = {
        "tbl": _pack_tbl(inputs),
        "wst1": wst1,
        "wsm": wsm,
        "wfc": wfc,
    }
    in_maps = []
    for c in range(N_CORES):
        sl = slice(c * NB, (c + 1) * NB)
        # [64, 17]: rows 0:32 = node 0 of each batch, rows 32:64 = node N-1
        t2 = trees[sl][:, [0, N - 1], :].transpose(1, 0, 2).reshape(NV, 17)
        li = np.zeros((NV, 2), np.int32)
        li[:, 0] = t2[:, 16]
        li[:, 1] = (np.arange(NV) % NB) * M
        in_maps.append(
            {
                "lstm": np.ascontiguousarray(lstm[sl].reshape(NB * M, LSTM_DIM)),
                "tok": _make_tok(t2),
                "li": li,
                **shared,
            }
        )
    return in_maps


_NC_CACHE = None


def run_on_hw(inputs, **kwargs):
    global _NC_CACHE
    if _NC_CACHE is None:
        _NC_CACHE = build_nc()
    in_maps = make_in_maps(inputs)
    return run_bass_kernel_spmd(
        _NC_CACHE, in_maps, core_ids=list(range(N_CORES)), **kwargs
    )


def kernel(**inputs) -> np.ndarray:
    res = run_on_hw(inputs)
    out = np.empty((B, 1), np.float32)
    for c in range(N_CORES):
        out[c * NB : (c + 1) * NB, 0] = res.results[c]["out"][0]
    return out
